# revision 32
# baseline (speedup 1.0000x reference)
"""Trainium2 Bass kernel for nn_CapsuleLayer (capsule layer: einsum + squash).

  u_hat = einsum('croi,bri->bcro', W[0], x)   # x:[256,1152,8] W:[1,10,1152,16,8]
  out   = squash(u_hat)                       # squash over last (o) axis

Strategy (8 NeuronCores, routes sharded 144/core, full batch per core),
default mode "f16":
  - Groups of 3 routes.  Per (q, batch-half) one 4-BANK psum tile [128,2048]
    whose four 512-col slices hold the four (kk, h) pairs:
      u-MM:  stationary x^T block [32=(3 routes x 8 in + pad), 128 batch],
             moving block-diagonal W [32, 480] -> slice[:, 0:480]
      sq-MM: stationary xx pair-products [128=(3 x 36 pairs + pad), 128 batch],
             moving block-diagonal sym-Gram cols [128, 30] -> slice[:, 480:510]
    where xx[b,(i,j)] = x_i*x_j (i<=j) and Gsym[(i,j),c] = (2-delta_ij)*G[i,j]
    with G = W_cr^T W_cr, so sq-MM emits sq_norm[b, (r,c)] = ||u||^2 directly.
    The 4-bank tile lets the scale chain read all four sq blocks with ONE
    strided ACT instruction (no per-bank copies).
  - All operands fp16 (error ~1.3e-3 vs the 2e-2 gate): halves DMA bytes and
    PE runs single-pass with fast weight loads.  ALL inputs are preloaded to
    SBUF once; the steady-state loop's only DMA is the fp16 output store.
  - Scale s = sqrt(sq)/(1+sq): t = ACT Sqrt (psum-strided read, one table
    set sqrt_and_others), d = ACT Identity(sq+1), r = DVE
    reciprocal_approx_fast(d) (~18 bits), s = Pool mul t*r (fp16).  ACT
    Rsqrt/Reciprocal are banned for accuracy.
  - PSUM eviction+scale (the real bottleneck: any PSUM-sourced op runs at
    ~1.3 ns/col on DVE or ACT, and Pool has no PSUM port) is split across
    engines per a measured 5:3 pattern:
      'A' units: DVE tensor_mul(ot, u_psum, s_bcast) directly  (~1.26us)
      'B' units: ACT copy psum->fp16 SBUF (~1.19us, frees the bank without
             waiting on s) + Pool bcast mul from SBUF (~1.89us)
  - Software-pipelined by one half-block: sq-MMs and the scale chain of
    half X are emitted BEFORE the wide unit ops of half X-1, so the chain's
    small DVE/Pool instructions sit ahead of wide muls in each engine's
    in-order queue and s is ready before its consumers issue.
  - Output DRAM layout [2*NQ, 128, 1920] mirrors the SBUF tiles: every
    store is contiguous 1920B-per-partition runs; host unpack restores
    [B, C, R, O].
"""

import sys

if "/opt/trn_rl_repo" not in sys.path:
    sys.path.insert(0, "/opt/trn_rl_repo")

from contextlib import ExitStack

import numpy as np

import concourse.bacc as bacc
import concourse.bass as bass
import concourse.mybir as mybir
import concourse.tile as tile
from concourse._compat import with_exitstack
from concourse.bass_utils import run_bass_kernel_spmd

# Problem shapes (hardcoded; harness provides full inputs)
B = 256          # batch
R = 1152         # num routes
C = 10           # num capsules
O = 16           # out channels
I = 8            # in channels
NCORES = 8
RL = R // NCORES                 # 144 routes per core
NG = RL // 3                     # 48 groups of 3 routes
NQ = NG // 4                     # 12 quad-blocks of 4 groups (row strips)
NPAIR = 36                       # i<=j pairs of 8 inputs
F32 = mybir.dt.float32
PAIRS = [(i, j) for i in range(I) for j in range(i, I)]


@with_exitstack
def _capsule_body(ctx: ExitStack, tc: "tile.TileContext",
                  out: bass.AP, xs: bass.AP, wm: bass.AP,
                  xxs: bass.AP, gs: bass.AP, reps: int = 1,
                  mode: str = "full"):
    nc = tc.nc

    if "fp32" in mode:
        mm_dt = F32
    elif "f16" in mode:
        # fp16 operands: half the loop-carried DMA bytes on wm/xxs, full-rate
        # single-pass PE, FWL on stationary loads; 10-bit mantissa keeps the
        # end-to-end error ~1e-3 (gate 2e-2).
        mm_dt = mybir.dt.float16
    else:
        mm_dt = mybir.dt.float32r
    # Optional: sq-path operands (xx pair products + gram cols) in bf16 —
    # halves the largest input tensor and enables FWL on the sq-matmul
    # stationary load; costs ~2x on the scale accuracy.
    sq_dt = mybir.dt.bfloat16 if "bxx" in mode else mm_dt
    # fp16 output store halves the dominant DMA stream (23.6MB -> 11.8MB per
    # core); elements are < 1 in magnitude so fp16 adds ~5e-4 relative error.
    out_dt = mybir.dt.float16 if "f16" in mode else F32

    singles = ctx.enter_context(tc.tile_pool(name="singles", bufs=1))
    # One 4-bank psum tile per (q, half): the four 512-col slices hold the
    # four (kk, h) matmul pairs, so the sq columns of all four land at a
    # fixed stride and ACT can read them in ONE strided instruction.
    psum_pool = ctx.enter_context(tc.tile_pool(name="psum", bufs=2, space="PSUM"))
    smalls = ctx.enter_context(tc.tile_pool(name="smalls", bufs=4))
    uev_pool = ctx.enter_context(tc.tile_pool(name="uev", bufs=6))
    out_pool = ctx.enter_context(tc.tile_pool(name="outs", bufs=8))

    # ALL inputs preloaded once (outside the reps loop): in fp16 everything
    # fits easily in SBUF (~6MB), so the steady-state loop's DMA traffic is
    # the output store alone.
    xs_sb = singles.tile([128, NQ * B], mm_dt)
    nc.gpsimd.dma_start(out=xs_sb[:], in_=xs.rearrange("p q b -> p (q b)"))
    gs_sb = singles.tile([128, NG * 30], sq_dt)
    nc.gpsimd.dma_start(out=gs_sb[:], in_=gs.rearrange("p g n -> p (g n)"))
    wm_sb = singles.tile([128, NQ * 480], mm_dt)
    nc.gpsimd.dma_start(out=wm_sb[:].rearrange("p (q v) -> p q v", q=NQ),
                        in_=wm.rearrange("q p v -> p q v"))
    xx_sb = singles.tile([128, NQ * 4 * B], sq_dt)
    nc.gpsimd.dma_start(out=xx_sb[:].rearrange("p (q n) -> p q n", q=NQ),
                        in_=xxs.rearrange("q p k b -> p q (k b)"))
    if "fakes" in mode:
        # Ablation: constant scale tile so the final muls run without the
        # scale chain feeding them.
        s_fake = singles.tile([128, 120], mybir.dt.float16)
        nc.vector.memset(s_fake[:], 0.5)

    if reps > 1:
        # Timing-only variant: run the whole body `reps` times on-device so
        # wall-clock differences cancel host/axon overhead.
        loop_cm = tc.For_i(0, reps, 1)
        ctx.enter_context(loop_cm)

    # Unit schedule: each (q, half) yields two merged [128, 960] output units
    # (one per h). Route each unit to an engine mix so DVE, ACT and Pool all
    # carry ~1/3 of the PSUM-eviction+scale work:
    #   'A'  — DVE direct:  tensor_mul(ot, u_psum, s_bcast)         ~1.26us
    #   'B'  — ACT evict (copy psum->fp16) + Pool bcast mul         ~1.19+1.89
    # Pattern: alternate A/B (8:8 — measured best balance point).
    unit_pat = ['A', 'B']
    unit_idx = 0

    def emit_units(ps, s_t, g0):
        # Merged output units: the two kk-groups for one h are adjacent
        # in DRAM (g, g+1), so ONE [128, 960] unit per h covers both u
        # slices (4-dim strided views across the psum banks); one merged
        # [128, 1920] store per half ships both h regions.
        nonlocal unit_idx
        ot = out_pool.tile([128, 1920], out_dt, tag="ot")
        for h in range(2):
            otv = (ot[:, 960 * h: 960 * h + 960]
                   .rearrange("p (g r c v) -> p g r c v", g=2, r=3, c=C))
            u_ap = (ps[:].rearrange("p (m x) -> p m x", m=4)
                    [:, h::2, 0:480]
                    .rearrange("p g (r c v) -> p g r c v", r=3, c=C))
            s_b = (s_t[:, 30 * h: 30 * h + 90]
                   .rearrange("p (a rc) -> p a rc", a=3)[:, ::2]
                   .rearrange("p g (r c) -> p g r c", r=3)
                   .unsqueeze(4).broadcast_to([128, 2, 3, C, O]))
            kind = unit_pat[unit_idx % len(unit_pat)]
            unit_idx += 1
            if kind == 'A':
                nc.vector.tensor_mul(otv, u_ap, s_b)
            else:
                # ACT evicts psum (frees the bank without waiting on s),
                # Pool applies the scale from SBUF.
                uev = uev_pool.tile([128, 960], out_dt, tag="uev")
                nc.scalar.copy(
                    uev[:].rearrange("p (g v) -> p g v", g=2),
                    ps[:].rearrange("p (m x) -> p m x", m=4)
                    [:, h::2, 0:480])
                nc.gpsimd.tensor_mul(
                    otv, uev[:].rearrange("p (g r c v) -> p g r c v",
                                          g=2, r=3, c=C), s_b)
            if "noout" not in mode:
                nc.sync.dma_start(
                    out=out[g0 // 2][:, 960 * h: 960 * h + 960],
                    in_=ot[:, 960 * h: 960 * h + 960])

    # Software pipelining by one half-block: emit half X's sq-MMs and scale
    # chain BEFORE half X-1's wide unit ops, so the chain's small DVE/Pool
    # instructions sit ahead of the previous half's wide muls in each
    # engine's in-order queue and s(X) is ready well before units(X) issue.
    pending = None
    # Per q: 4 groups stacked on the 4 row strips (partition blocks of 32);
    # two half-blocks of 2 groups x 2 batch-halves land in one 4-bank psum
    # tile each; psum bufs=2 double-buffers half-blocks.
    for q in range(NQ):
        wm_t = wm_sb[:, q * 480:(q + 1) * 480]
        xx_t = xx_sb[:, q * 4 * B:(q + 1) * 4 * B]
        for half in range(2):
            # 4-bank psum tile; slice m = 2*kk + h holds (u, sq) of that pair.
            ps = psum_pool.tile([128, 2048], F32, tag="ps")
            quads = []
            for kk in range(2):
                k = 2 * half + kk
                g = 4 * q + k
                for h in range(2):
                    quads.append((2 * kk + h, g, h))
            # sq-MMs FIRST: they are tiny (30 moving cols each), so the last
            # sq lands early and the scale chain runs concurrently with the
            # four wide u-MMs instead of after them.
            for m, g, h in quads:
                k = g - 4 * q
                nc.tensor.matmul(
                    ps[:, 512 * m + 480: 512 * m + 510],
                    xx_t[:, k * B + h * 128: k * B + h * 128 + 128],
                    gs_sb[:, g * 30: g * 30 + 30], start=True, stop=True,
                    tile_position=(0, 0))

            if "nosquash" not in mode:
                if "fakes" in mode:
                    s_t = s_fake
                else:
                    # Scale chain once per half-block on the strided sq view
                    # (ACT reads across the four banks — no copies):
                    #   s = sqrt(sq) / (1 + sq)
                    # t = Sqrt(sq) [ACT, sqrt table]; d = sq + 1 [ACT
                    # Identity with bias]; r = 1/d [DVE recip_approx_fast,
                    # ~18 bits]; s = t*r [Pool, fp16 out]. One activation-
                    # table set (sqrt_and_others), tiny DVE/Pool cost.
                    sq_v = (ps[:].rearrange("p (m x) -> p m x", m=4)
                            [:, :, 480:510])
                    t_t = smalls.tile([128, 120], F32, tag="t")
                    nc.scalar.activation(
                        t_t[:].rearrange("p (m n) -> p m n", m=4), sq_v,
                        mybir.ActivationFunctionType.Sqrt)
                    d_t = smalls.tile([128, 120], F32, tag="d")
                    nc.scalar.activation(
                        d_t[:].rearrange("p (m n) -> p m n", m=4), sq_v,
                        mybir.ActivationFunctionType.Identity, bias=1.0)
                    r_t = smalls.tile([128, 120], F32, tag="r")
                    nc.vector.reciprocal_approx_fast(out=r_t[:], in_=d_t[:])
                    s_t = smalls.tile([128, 120], mybir.dt.float16, tag="s")
                    nc.gpsimd.tensor_mul(s_t[:], t_t[:], r_t[:])

                if pending is not None and "nomul" not in mode:
                    emit_units(*pending)
                pending = (ps, s_t, 4 * q + 2 * half)

            for m, g, h in quads:
                k = 2 * half + (m // 2)
                nc.tensor.matmul(
                    ps[:, 512 * m: 512 * m + 480],
                    xs_sb[32 * k:32 * k + 32,
                          q * B + h * 128: q * B + h * 128 + 128],
                    wm_t[32 * k:32 * k + 32, :], start=True, stop=True,
                    tile_position=(32 * k, 0))

    if pending is not None and "nomul" not in mode:
        emit_units(*pending)


def build_bass(reps: int = 1, mode: str = "full"):
    # Bacc (not plain Bass): its compile() runs generate_event_semaphores,
    # which splits multi-semaphore waits — TPB instructions carry only one
    # wait slot in hardware — plus move_matmul_waits_to_ldweights etc.
    nc = bacc.Bacc("TRN2", target_bir_lowering=False, debug=False,
                   num_devices=NCORES)
    if "fp32" in mode:
        in_dt = F32
    elif "f16" in mode:
        in_dt = mybir.dt.float16
    else:
        in_dt = mybir.dt.float32r
    sq_in_dt = mybir.dt.bfloat16 if "bxx" in mode else in_dt
    out_dram_dt = mybir.dt.float16 if "f16" in mode else F32
    xs = nc.dram_tensor("xs", [128, NQ, B], in_dt, kind="ExternalInput")
    wm = nc.dram_tensor("wm", [NQ, 128, 480], in_dt, kind="ExternalInput")
    xxs = nc.dram_tensor("xxs", [NQ, 128, 4, B], sq_in_dt, kind="ExternalInput")
    gs = nc.dram_tensor("gs", [128, NG, 30], sq_in_dt, kind="ExternalInput")
    # One row per half-block (2q+half): [128, (h2, g2, 480)] — matches the
    # merged SBUF output tile exactly so each store is a single contiguous
    # 3840B-per-partition DMA.
    out = nc.dram_tensor("out", [2 * NQ, 128, 1920], out_dram_dt,
                         kind="ExternalOutput")
    with tile.TileContext(nc) as tc:
        _capsule_body(tc, out[:], xs[:], wm[:], xxs[:], gs[:],
                      reps=reps, mode=mode)

    # All ACT functions used here (Sqrt, Identity, Copy) coexist in the
    # sqrt_and_others table set, but the stock table-load pass assigns each
    # function its *first* containing set, alternating sets and inserting
    # ~2.7us table loads throughout.  Strip our functions from all other
    # sets (keeping positional act_func_set ids intact) so resolution lands
    # on the one set and a single load is emitted.
    import types
    from concourse.hw_specs import get_activation_tables
    from concourse import bacc as _bacc_mod

    _PIN = "sqrt_and_others"
    _FUNCS = {mybir.ActivationFunctionType.Square,
              mybir.ActivationFunctionType.Sqrt,
              mybir.ActivationFunctionType.Copy,
              mybir.ActivationFunctionType.Identity}

    def _one_set_table_loads(self):
        tables = [
            (k, (v if k == _PIN else (v - _FUNCS)))
            for k, v in get_activation_tables(self.m.arch).items()
        ]
        _bacc_mod._bass_rust.insert_act_table_loads(self, tables)

    nc.insert_act_table_loads = types.MethodType(_one_set_table_loads, nc)
    nc.compile()
    return nc


_NC = {}


def _get_nc(reps: int = 1, mode: str = "full"):
    key = (reps, mode)
    if key not in _NC:
        _NC[key] = build_bass(reps, mode)
    return _NC[key]


def _pack_inputs(x: np.ndarray, W: np.ndarray):
    """Build per-core xs [32,48,256], wm [48,32,480], xxs [48,128,256],
    gs [48,128,30]."""
    x = np.ascontiguousarray(x, dtype=np.float32)
    W0 = np.ascontiguousarray(W.reshape(C, R, O, I), dtype=np.float32)

    # x stationaries: [R, I, B] -> rows padded to 32, 4 groups stacked on the
    # 128 partitions (full-width DMA): [cores, 128=(k,row), NQ, B]
    xt = x.transpose(1, 2, 0)                        # [R, I, B]
    xs = np.zeros((NCORES, NG, 32, B), np.float32)
    xs[:, :, :24] = xt.reshape(NCORES, NG, 24, B)
    xs = xs.reshape(NCORES, NQ, 4, 32, B).transpose(0, 2, 3, 1, 4)
    xs = np.ascontiguousarray(xs.reshape(NCORES, 128, NQ, B))

    # W moving blocks, 4 groups stacked on partitions: [cores, NQ, 128, 480]
    Wt = W0.transpose(1, 3, 0, 2)                    # [R, I, C, O]
    Wt = Wt.reshape(NCORES, NG, 3, I, C * O)         # k,g,r,i,co
    wm = np.zeros((NCORES, NG, 32, 3, C * O), np.float32)
    for r in range(3):
        wm[:, :, r * I:(r + 1) * I, r] = Wt[:, :, r]
    wm = np.ascontiguousarray(wm.reshape(NCORES, NQ, 128, 480))

    # xx pair products: [B, R, 36] -> [cores, NQ, 4, (3*36 padded 128), B]
    ii = np.array([p[0] for p in PAIRS])
    jj = np.array([p[1] for p in PAIRS])
    xx = x[:, :, ii] * x[:, :, jj]                   # [B, R, 36]
    xxt = xx.transpose(1, 2, 0)                      # [R, 36, B]
    xxs = np.zeros((NCORES, NG, 128, B), np.float32)
    xxs[:, :, :108] = xxt.reshape(NCORES, NG, 108, B)
    xxs = np.ascontiguousarray(
        xxs.reshape(NCORES, NQ, 4, 128, B).transpose(0, 1, 3, 2, 4))

    # Gram columns: [cores, 48, 128, 30] block-diagonal over the 3 routes
    W64 = W0.astype(np.float64)
    G = np.einsum('croi,croj->crij', W64, W64)       # [C, R, I, I]
    Gsym = G[:, :, ii, jj] * np.where(ii == jj, 1.0, 2.0)   # [C, R, 36]
    Gt = Gsym.transpose(1, 2, 0).astype(np.float32)  # [R, 36, C]
    Gt = Gt.reshape(NCORES, NG, 3, NPAIR, C)
    gs = np.zeros((NCORES, NG, 128, 30), np.float32)
    for r in range(3):
        gs[:, :, r * NPAIR:(r + 1) * NPAIR, r * C:(r + 1) * C] = Gt[:, :, r]
    gs = np.ascontiguousarray(gs.transpose(0, 2, 1, 3))   # [cores, 128, 48, 30]
    return xs, wm, xxs, gs


def _unpack_outputs(results):
    """Per-core out [2*NQ, 128, 1920] -> full [B, C, R, O]."""
    full = np.empty((B, C, R, O), dtype=np.float32)
    for k in range(NCORES):
        # dims: hb, p, h, g, r, c, o ; b = 128h + p ; route = 3*(2hb+g) + r
        ok = results[k]["out"].reshape(2 * NQ, 128, 2, 2, 3, C, O)
        fk = ok.transpose(2, 1, 5, 0, 3, 4, 6).reshape(B, C, RL, O)
        full[:, :, k * RL:(k + 1) * RL, :] = fk
    return full


def _make_in_maps(packed, mode: str = "full"):
    xs, wm, xxs, gs = packed
    if "f16" in mode:
        xs = xs.astype(np.float16)
        wm = wm.astype(np.float16)
        xxs = xxs.astype(np.float16)
        gs = gs.astype(np.float16)
    elif "bxx" in mode:
        import ml_dtypes
        xxs = xxs.astype(ml_dtypes.bfloat16)
        gs = gs.astype(ml_dtypes.bfloat16)
    return [{"xs": xs[k], "wm": wm[k], "xxs": xxs[k], "gs": gs[k]}
            for k in range(NCORES)]


def run_packed(packed, reps: int = 1, mode: str = "full"):
    nc = _get_nc(reps, mode)
    in_maps = _make_in_maps(packed, mode)
    return run_bass_kernel_spmd(nc, in_maps, list(range(NCORES)))


def kernel(x: np.ndarray, W: np.ndarray, **_ignored):
    x = np.asarray(x, dtype=np.float32)
    W = np.asarray(W, dtype=np.float32)
    assert x.shape == (B, R, I), x.shape
    packed = _pack_inputs(x, W)
    res = run_packed(packed, mode="f16")
    return _unpack_outputs(res.results)



# revision 33
# speedup vs baseline: 1.2511x; 1.2511x over previous
"""Trainium2 Bass kernel for nn_CapsuleLayer (capsule layer: einsum + squash).

  u_hat = einsum('croi,bri->bcro', W[0], x)   # x:[256,1152,8] W:[1,10,1152,16,8]
  out   = squash(u_hat)                       # squash over last (o) axis

Strategy (8 NeuronCores, routes sharded 144/core, full batch per core),
default mode "f16":
  - Groups of 3 routes.  Per (q, batch-half) one 4-BANK psum tile [128,2048]
    whose four 512-col slices hold the four (kk, h) pairs:
      u-MM:  stationary x^T block [32=(3 routes x 8 in + pad), 128 batch],
             moving block-diagonal W [32, 480] -> slice[:, 0:480]
      sq-MM: stationary xx pair-products [128=(3 x 36 pairs + pad), 128 batch],
             moving block-diagonal sym-Gram cols [128, 30] -> slice[:, 480:510]
    where xx[b,(i,j)] = x_i*x_j (i<=j) and Gsym[(i,j),c] = (2-delta_ij)*G[i,j]
    with G = W_cr^T W_cr, so sq-MM emits sq_norm[b, (r,c)] = ||u||^2 directly.
    The 4-bank tile lets the scale chain read all four sq blocks with ONE
    strided ACT instruction (no per-bank copies).
  - All operands fp16 (error ~1.3e-3 vs the 2e-2 gate): halves DMA bytes and
    PE runs single-pass with fast weight loads.  ALL inputs are preloaded to
    SBUF once; the steady-state loop's only DMA is the fp16 output store.
  - Scale s = sqrt(sq)/(1+sq): t = ACT Sqrt (psum-strided read, one table
    set sqrt_and_others), d = ACT Identity(sq+1), r = DVE
    reciprocal_approx_fast(d) (~18 bits), s = Pool mul t*r (fp16).  ACT
    Rsqrt/Reciprocal are banned for accuracy.
  - PSUM eviction+scale (the real bottleneck: any PSUM-sourced op runs at
    ~1.3 ns/col on DVE or ACT, and Pool has no PSUM port) is split across
    engines per a measured 5:3 pattern:
      'A' units: DVE tensor_mul(ot, u_psum, s_bcast) directly  (~1.26us)
      'B' units: ACT copy psum->fp16 SBUF (~1.19us, frees the bank without
             waiting on s) + Pool bcast mul from SBUF (~1.89us)
  - Software-pipelined by one half-block: sq-MMs and the scale chain of
    half X are emitted BEFORE the wide unit ops of half X-1, so the chain's
    small DVE/Pool instructions sit ahead of wide muls in each engine's
    in-order queue and s is ready before its consumers issue.
  - Output DRAM layout [2*NQ, 128, 1920] mirrors the SBUF tiles: every
    store is contiguous 1920B-per-partition runs; host unpack restores
    [B, C, R, O].
"""

import sys

if "/opt/trn_rl_repo" not in sys.path:
    sys.path.insert(0, "/opt/trn_rl_repo")

from contextlib import ExitStack

import numpy as np

import concourse.bacc as bacc
import concourse.bass as bass
import concourse.mybir as mybir
import concourse.tile as tile
from concourse._compat import with_exitstack
from concourse.bass_utils import run_bass_kernel_spmd

# Problem shapes (hardcoded; harness provides full inputs)
B = 256          # batch
R = 1152         # num routes
C = 10           # num capsules
O = 16           # out channels
I = 8            # in channels
NCORES = 8
RL = R // NCORES                 # 144 routes per core
NG = RL // 3                     # 48 groups of 3 routes
NQ = NG // 4                     # 12 quad-blocks of 4 groups (row strips)
NPAIR = 36                       # i<=j pairs of 8 inputs
F32 = mybir.dt.float32
PAIRS = [(i, j) for i in range(I) for j in range(i, I)]


@with_exitstack
def _capsule_body(ctx: ExitStack, tc: "tile.TileContext",
                  out: bass.AP, xs: bass.AP, wm: bass.AP,
                  xxs: bass.AP, gs: bass.AP, reps: int = 1,
                  mode: str = "full"):
    nc = tc.nc

    if "fp32" in mode:
        mm_dt = F32
    elif "f16" in mode:
        # fp16 operands: half the loop-carried DMA bytes on wm/xxs, full-rate
        # single-pass PE, FWL on stationary loads; 10-bit mantissa keeps the
        # end-to-end error ~1e-3 (gate 2e-2).
        mm_dt = mybir.dt.float16
    else:
        mm_dt = mybir.dt.float32r
    # Optional: sq-path operands (xx pair products + gram cols) in bf16 —
    # halves the largest input tensor and enables FWL on the sq-matmul
    # stationary load; costs ~2x on the scale accuracy.
    sq_dt = mybir.dt.bfloat16 if "bxx" in mode else mm_dt
    # fp16 output store halves the dominant DMA stream (23.6MB -> 11.8MB per
    # core); elements are < 1 in magnitude so fp16 adds ~5e-4 relative error.
    out_dt = mybir.dt.float16 if "f16" in mode else F32

    singles = ctx.enter_context(tc.tile_pool(name="singles", bufs=1))
    # One 4-bank psum tile per (q, half): the four 512-col slices hold the
    # four (kk, h) matmul pairs, so the sq columns of all four land at a
    # fixed stride and ACT can read them in ONE strided instruction.
    psum_pool = ctx.enter_context(tc.tile_pool(name="psum", bufs=2, space="PSUM"))
    smalls = ctx.enter_context(tc.tile_pool(name="smalls", bufs=4))
    uev_pool = ctx.enter_context(tc.tile_pool(name="uev", bufs=6))
    out_pool = ctx.enter_context(tc.tile_pool(name="outs", bufs=8))

    # ALL inputs preloaded once (outside the reps loop): in fp16 everything
    # fits easily in SBUF (~6MB), so the steady-state loop's DMA traffic is
    # the output store alone.
    xs_sb = singles.tile([128, NQ * B], mm_dt)
    nc.gpsimd.dma_start(out=xs_sb[:], in_=xs.rearrange("p q b -> p (q b)"))
    gs_sb = singles.tile([128, NG * 30], sq_dt)
    nc.gpsimd.dma_start(out=gs_sb[:], in_=gs.rearrange("p g n -> p (g n)"))
    wm_sb = singles.tile([128, NQ * 480], mm_dt)
    nc.gpsimd.dma_start(out=wm_sb[:].rearrange("p (q v) -> p q v", q=NQ),
                        in_=wm.rearrange("q p v -> p q v"))
    xx_sb = singles.tile([128, NQ * 4 * B], sq_dt)
    nc.gpsimd.dma_start(out=xx_sb[:].rearrange("p (q n) -> p q n", q=NQ),
                        in_=xxs.rearrange("q p k b -> p q (k b)"))
    if "fakes" in mode:
        # Ablation: constant scale tile so the final muls run without the
        # scale chain feeding them.
        s_fake = singles.tile([128, 120], mybir.dt.float16)
        nc.vector.memset(s_fake[:], 0.5)

    if reps > 1:
        # Timing-only variant: run the whole body `reps` times on-device so
        # wall-clock differences cancel host/axon overhead.
        loop_cm = tc.For_i(0, reps, 1)
        ctx.enter_context(loop_cm)

    # Unit schedule: each (q, half) yields two merged [128, 960] output units
    # (one per h). Route each unit to an engine mix so DVE, ACT and Pool all
    # carry ~1/3 of the PSUM-eviction+scale work:
    #   'A'  — DVE direct:  tensor_mul(ot, u_psum, s_bcast)         ~1.26us
    #   'B'  — ACT evict (copy psum->fp16) + Pool bcast mul         ~1.19+1.89
    # Pattern of 16 units: 9 A + 7 B (measured balance point; 5A/3B gave
    # 71.1us, 8A/8B 85.4us, 9A/7B 68.9us).
    unit_pat = ['A', 'B', 'A', 'A', 'B', 'A', 'A', 'B',
                'A', 'B', 'A', 'B', 'A', 'B', 'A', 'B']
    unit_idx = 0

    def emit_units(ps, s_t, g0):
        # Merged output units: the two kk-groups for one h are adjacent
        # in DRAM (g, g+1), so ONE [128, 960] unit per h covers both u
        # slices (4-dim strided views across the psum banks); one merged
        # [128, 1920] store per half ships both h regions.
        nonlocal unit_idx
        ot = out_pool.tile([128, 1920], out_dt, tag="ot")
        for h in range(2):
            otv = (ot[:, 960 * h: 960 * h + 960]
                   .rearrange("p (g r c v) -> p g r c v", g=2, r=3, c=C))
            u_ap = (ps[:].rearrange("p (m x) -> p m x", m=4)
                    [:, h::2, 0:480]
                    .rearrange("p g (r c v) -> p g r c v", r=3, c=C))
            s_b = (s_t[:, 30 * h: 30 * h + 90]
                   .rearrange("p (a rc) -> p a rc", a=3)[:, ::2]
                   .rearrange("p g (r c) -> p g r c", r=3)
                   .unsqueeze(4).broadcast_to([128, 2, 3, C, O]))
            kind = unit_pat[unit_idx % len(unit_pat)]
            unit_idx += 1
            if kind == 'A':
                nc.vector.tensor_mul(otv, u_ap, s_b)
            else:
                # ACT evicts psum (frees the bank without waiting on s),
                # Pool applies the scale from SBUF.
                uev = uev_pool.tile([128, 960], out_dt, tag="uev")
                nc.scalar.copy(
                    uev[:].rearrange("p (g v) -> p g v", g=2),
                    ps[:].rearrange("p (m x) -> p m x", m=4)
                    [:, h::2, 0:480])
                nc.gpsimd.tensor_mul(
                    otv, uev[:].rearrange("p (g r c v) -> p g r c v",
                                          g=2, r=3, c=C), s_b)
            if "noout" not in mode:
                nc.sync.dma_start(
                    out=out[g0 // 2][:, 960 * h: 960 * h + 960],
                    in_=ot[:, 960 * h: 960 * h + 960])

    # Software pipelining by one half-block: emit half X's sq-MMs and scale
    # chain BEFORE half X-1's wide unit ops, so the chain's small DVE/Pool
    # instructions sit ahead of the previous half's wide muls in each
    # engine's in-order queue and s(X) is ready well before units(X) issue.
    pending = None
    # Per q: 4 groups stacked on the 4 row strips (partition blocks of 32);
    # two half-blocks of 2 groups x 2 batch-halves land in one 4-bank psum
    # tile each; psum bufs=2 double-buffers half-blocks.
    for q in range(NQ):
        wm_t = wm_sb[:, q * 480:(q + 1) * 480]
        xx_t = xx_sb[:, q * 4 * B:(q + 1) * 4 * B]
        for half in range(2):
            # 4-bank psum tile; slice m = 2*kk + h holds (u, sq) of that pair.
            ps = psum_pool.tile([128, 2048], F32, tag="ps")
            quads = []
            for kk in range(2):
                k = 2 * half + kk
                g = 4 * q + k
                for h in range(2):
                    quads.append((2 * kk + h, g, h))
            # sq-MMs FIRST: they are tiny (30 moving cols each), so the last
            # sq lands early and the scale chain runs concurrently with the
            # four wide u-MMs instead of after them.
            for m, g, h in quads:
                k = g - 4 * q
                nc.tensor.matmul(
                    ps[:, 512 * m + 480: 512 * m + 510],
                    xx_t[:, k * B + h * 128: k * B + h * 128 + 128],
                    gs_sb[:, g * 30: g * 30 + 30], start=True, stop=True,
                    tile_position=(0, 0))

            if "nosquash" not in mode:
                if "fakes" in mode:
                    s_t = s_fake
                else:
                    # Scale chain once per half-block on the strided sq view
                    # (ACT reads across the four banks — no copies):
                    #   s = sqrt(sq) / (1 + sq)
                    # t = Sqrt(sq) [ACT, sqrt table]; d = sq + 1 [ACT
                    # Identity with bias]; r = 1/d [DVE recip_approx_fast,
                    # ~18 bits]; s = t*r [Pool, fp16 out]. One activation-
                    # table set (sqrt_and_others), tiny DVE/Pool cost.
                    sq_v = (ps[:].rearrange("p (m x) -> p m x", m=4)
                            [:, :, 480:510])
                    t_t = smalls.tile([128, 120], F32, tag="t")
                    nc.scalar.activation(
                        t_t[:].rearrange("p (m n) -> p m n", m=4), sq_v,
                        mybir.ActivationFunctionType.Sqrt)
                    d_t = smalls.tile([128, 120], F32, tag="d")
                    nc.scalar.activation(
                        d_t[:].rearrange("p (m n) -> p m n", m=4), sq_v,
                        mybir.ActivationFunctionType.Identity, bias=1.0)
                    r_t = smalls.tile([128, 120], F32, tag="r")
                    nc.vector.reciprocal_approx_fast(out=r_t[:], in_=d_t[:])
                    s_t = smalls.tile([128, 120], mybir.dt.float16, tag="s")
                    nc.gpsimd.tensor_mul(s_t[:], t_t[:], r_t[:])

                if pending is not None and "nomul" not in mode:
                    emit_units(*pending)
                pending = (ps, s_t, 4 * q + 2 * half)

            for m, g, h in quads:
                k = 2 * half + (m // 2)
                nc.tensor.matmul(
                    ps[:, 512 * m: 512 * m + 480],
                    xs_sb[32 * k:32 * k + 32,
                          q * B + h * 128: q * B + h * 128 + 128],
                    wm_t[32 * k:32 * k + 32, :], start=True, stop=True,
                    tile_position=(32 * k, 0))

    if pending is not None and "nomul" not in mode:
        emit_units(*pending)


def build_bass(reps: int = 1, mode: str = "full"):
    # Bacc (not plain Bass): its compile() runs generate_event_semaphores,
    # which splits multi-semaphore waits — TPB instructions carry only one
    # wait slot in hardware — plus move_matmul_waits_to_ldweights etc.
    nc = bacc.Bacc("TRN2", target_bir_lowering=False, debug=False,
                   num_devices=NCORES)
    if "fp32" in mode:
        in_dt = F32
    elif "f16" in mode:
        in_dt = mybir.dt.float16
    else:
        in_dt = mybir.dt.float32r
    sq_in_dt = mybir.dt.bfloat16 if "bxx" in mode else in_dt
    out_dram_dt = mybir.dt.float16 if "f16" in mode else F32
    xs = nc.dram_tensor("xs", [128, NQ, B], in_dt, kind="ExternalInput")
    wm = nc.dram_tensor("wm", [NQ, 128, 480], in_dt, kind="ExternalInput")
    xxs = nc.dram_tensor("xxs", [NQ, 128, 4, B], sq_in_dt, kind="ExternalInput")
    gs = nc.dram_tensor("gs", [128, NG, 30], sq_in_dt, kind="ExternalInput")
    # One row per half-block (2q+half): [128, (h2, g2, 480)] — matches the
    # merged SBUF output tile exactly so each store is a single contiguous
    # 3840B-per-partition DMA.
    out = nc.dram_tensor("out", [2 * NQ, 128, 1920], out_dram_dt,
                         kind="ExternalOutput")
    with tile.TileContext(nc) as tc:
        _capsule_body(tc, out[:], xs[:], wm[:], xxs[:], gs[:],
                      reps=reps, mode=mode)

    # All ACT functions used here (Sqrt, Identity, Copy) coexist in the
    # sqrt_and_others table set, but the stock table-load pass assigns each
    # function its *first* containing set, alternating sets and inserting
    # ~2.7us table loads throughout.  Strip our functions from all other
    # sets (keeping positional act_func_set ids intact) so resolution lands
    # on the one set and a single load is emitted.
    import types
    from concourse.hw_specs import get_activation_tables
    from concourse import bacc as _bacc_mod

    _PIN = "sqrt_and_others"
    _FUNCS = {mybir.ActivationFunctionType.Square,
              mybir.ActivationFunctionType.Sqrt,
              mybir.ActivationFunctionType.Copy,
              mybir.ActivationFunctionType.Identity}

    def _one_set_table_loads(self):
        tables = [
            (k, (v if k == _PIN else (v - _FUNCS)))
            for k, v in get_activation_tables(self.m.arch).items()
        ]
        _bacc_mod._bass_rust.insert_act_table_loads(self, tables)

    nc.insert_act_table_loads = types.MethodType(_one_set_table_loads, nc)
    nc.compile()
    return nc


_NC = {}


def _get_nc(reps: int = 1, mode: str = "full"):
    key = (reps, mode)
    if key not in _NC:
        _NC[key] = build_bass(reps, mode)
    return _NC[key]


def _pack_inputs(x: np.ndarray, W: np.ndarray):
    """Build per-core xs [32,48,256], wm [48,32,480], xxs [48,128,256],
    gs [48,128,30]."""
    x = np.ascontiguousarray(x, dtype=np.float32)
    W0 = np.ascontiguousarray(W.reshape(C, R, O, I), dtype=np.float32)

    # x stationaries: [R, I, B] -> rows padded to 32, 4 groups stacked on the
    # 128 partitions (full-width DMA): [cores, 128=(k,row), NQ, B]
    xt = x.transpose(1, 2, 0)                        # [R, I, B]
    xs = np.zeros((NCORES, NG, 32, B), np.float32)
    xs[:, :, :24] = xt.reshape(NCORES, NG, 24, B)
    xs = xs.reshape(NCORES, NQ, 4, 32, B).transpose(0, 2, 3, 1, 4)
    xs = np.ascontiguousarray(xs.reshape(NCORES, 128, NQ, B))

    # W moving blocks, 4 groups stacked on partitions: [cores, NQ, 128, 480]
    Wt = W0.transpose(1, 3, 0, 2)                    # [R, I, C, O]
    Wt = Wt.reshape(NCORES, NG, 3, I, C * O)         # k,g,r,i,co
    wm = np.zeros((NCORES, NG, 32, 3, C * O), np.float32)
    for r in range(3):
        wm[:, :, r * I:(r + 1) * I, r] = Wt[:, :, r]
    wm = np.ascontiguousarray(wm.reshape(NCORES, NQ, 128, 480))

    # xx pair products: [B, R, 36] -> [cores, NQ, 4, (3*36 padded 128), B]
    ii = np.array([p[0] for p in PAIRS])
    jj = np.array([p[1] for p in PAIRS])
    xx = x[:, :, ii] * x[:, :, jj]                   # [B, R, 36]
    xxt = xx.transpose(1, 2, 0)                      # [R, 36, B]
    xxs = np.zeros((NCORES, NG, 128, B), np.float32)
    xxs[:, :, :108] = xxt.reshape(NCORES, NG, 108, B)
    xxs = np.ascontiguousarray(
        xxs.reshape(NCORES, NQ, 4, 128, B).transpose(0, 1, 3, 2, 4))

    # Gram columns: [cores, 48, 128, 30] block-diagonal over the 3 routes
    W64 = W0.astype(np.float64)
    G = np.einsum('croi,croj->crij', W64, W64)       # [C, R, I, I]
    Gsym = G[:, :, ii, jj] * np.where(ii == jj, 1.0, 2.0)   # [C, R, 36]
    Gt = Gsym.transpose(1, 2, 0).astype(np.float32)  # [R, 36, C]
    Gt = Gt.reshape(NCORES, NG, 3, NPAIR, C)
    gs = np.zeros((NCORES, NG, 128, 30), np.float32)
    for r in range(3):
        gs[:, :, r * NPAIR:(r + 1) * NPAIR, r * C:(r + 1) * C] = Gt[:, :, r]
    gs = np.ascontiguousarray(gs.transpose(0, 2, 1, 3))   # [cores, 128, 48, 30]
    return xs, wm, xxs, gs


def _unpack_outputs(results):
    """Per-core out [2*NQ, 128, 1920] -> full [B, C, R, O]."""
    full = np.empty((B, C, R, O), dtype=np.float32)
    for k in range(NCORES):
        # dims: hb, p, h, g, r, c, o ; b = 128h + p ; route = 3*(2hb+g) + r
        ok = results[k]["out"].reshape(2 * NQ, 128, 2, 2, 3, C, O)
        fk = ok.transpose(2, 1, 5, 0, 3, 4, 6).reshape(B, C, RL, O)
        full[:, :, k * RL:(k + 1) * RL, :] = fk
    return full


def _make_in_maps(packed, mode: str = "full"):
    xs, wm, xxs, gs = packed
    if "f16" in mode:
        xs = xs.astype(np.float16)
        wm = wm.astype(np.float16)
        xxs = xxs.astype(np.float16)
        gs = gs.astype(np.float16)
    elif "bxx" in mode:
        import ml_dtypes
        xxs = xxs.astype(ml_dtypes.bfloat16)
        gs = gs.astype(ml_dtypes.bfloat16)
    return [{"xs": xs[k], "wm": wm[k], "xxs": xxs[k], "gs": gs[k]}
            for k in range(NCORES)]


def run_packed(packed, reps: int = 1, mode: str = "full"):
    nc = _get_nc(reps, mode)
    in_maps = _make_in_maps(packed, mode)
    return run_bass_kernel_spmd(nc, in_maps, list(range(NCORES)))


def kernel(x: np.ndarray, W: np.ndarray, **_ignored):
    x = np.asarray(x, dtype=np.float32)
    W = np.asarray(W, dtype=np.float32)
    assert x.shape == (B, R, I), x.shape
    packed = _pack_inputs(x, W)
    res = run_packed(packed, mode="f16")
    return _unpack_outputs(res.results)

